# revision 85
# baseline (speedup 1.0000x reference)
"""Constraint-projection layer on 8 Trainium2 NeuronCores.

Reference computes, per batch row y_i:  x_i = argmin ||x - y_i|| s.t. A x = b_i
via a dense KKT solve. Closed form (Schur complement of the KKT system):

    x = y - A^T (A A^T)^{-1} (A y - b)

Host precomputes W = (A A^T)^{-1} A  (128 x 1024, float64 solve). Device I/O
is fp16 (b in fp8 e4m3) — the kernel is DMA-bound, so halving wire bytes
halves its runtime, and fp16 runs the PE at 1 cycle/row instead of fp32's 4.
Accumulation stays fp32 in PSUM; measured rel err ~6.7e-4 vs the 2e-2 gate.

Each core gets a 2048-row batch shard in TRANSPOSED layout (dim-major):

    stage 1:  T^T = A @ Y^T - B^T          (128 m  x 2048 batch)
    stage 2:  X^T = Y^T - W^T @ T^T        (1024 d x 2048 batch)

Stage 2 runs per 128-dim d-chunk, alternating consumers so no one engine
gates the DMA stream: even chunks do PSUM = W^T T then DVE computes
y - PSUM (fp16 out); odd chunks (W uploaded negated) accumulate
PSUM = (-W)^T T + I^T Y on the PE so the Activation engine only
copies/downcasts PSUM -> SBUF. Stores stream per chunk-pair. The schedule
is tuned against the TimelineSim cost model: the DMA engines run gapless
from first load to last store (~25.6us of fp16 traffic at 360 B/ns), with
a ~2.3us front pipe-fill and ~1.4us drain tail around it.

Data-parallel: no cross-core communication.
"""

import os

import numpy as np
import bass_rust as _br
import concourse.bass as bass
import concourse.mybir as mybir
from concourse import tile
from concourse.bass_utils import run_bass_kernel_spmd

F32 = mybir.dt.float32
F8 = mybir.dt.float8e4  # e4m3: carries b, whose error reaches x through
                        # A^T(AA^T)^{-1} with gain ~1/20 — ~4e-4 rel worst

# I/O dtype: fp16 default; bf16 fallback switch kept for HW-compile issues.
IO_MODE = os.environ.get("KERNEL_IO_DTYPE", "f16")
if IO_MODE == "bf16":
    F16 = mybir.dt.bfloat16
else:
    F16 = mybir.dt.float16


def _np_f16():
    if IO_MODE == "bf16":
        import ml_dtypes

        return np.dtype(ml_dtypes.bfloat16)
    return np.dtype(np.float16)

N_CORES = 8
BATCH = 16384
N = 1024           # input dim
M = 128            # constraint dim
BC = BATCH // N_CORES  # 2048 batch rows per core
KC = N // 128      # 8 contraction chunks
F = 512            # free-dim tile (one PSUM bank of f32)
NJ = BC // F       # 4 batch tiles per core
N_WARM = 6         # dummy matmuls that pre-ramp the PE pstate clock; the
                   # at-derivation matmuls continue the ramp after these


def _split_drain_and_barrier(self, tick_clock, wait_clock):
    # Slim Tile tail. The stock sequence is nops+drain+barrier+clears+
    # barrier; here the final sem waits ride the Pool engine (which also
    # owns the sem clears, so program order alone sequences wait->clear),
    # the barriers go away entirely, and only SP drains its DGE ring —
    # every DMA in this kernel is issued from the sync ring. One wait per
    # nop because walrus rejects >2 sync waits per instruction
    # (CTRL_NO_STRUCT). The Pool nops waiting all engines' final sem values
    # (including every DMA-completion sem) guarantees outputs are in DRAM
    # and no engine still touches a semaphore when the clears run.
    gc = tick_clock.global_clock
    vals = eval(repr(gc).replace("VectorClock", "").strip("()"))
    # Wait in each semaphore's firing order (last-updater emission position
    # tracks completion on the in-order DMA queue), so only the final
    # store's completion sem sits on the critical path — the other nops
    # retire while it is still propagating.
    num_to_idx = {v.num: k for k, v in self.sems.allocated().items()}
    last_pos = {}
    pos = 0
    for blk in self.nc.m.functions[0].blocks:
        for ins in blk.instructions:
            si = getattr(ins, "sync_info", None)
            if si is not None:
                for u in si.on_update:
                    if u.id in num_to_idx:
                        last_pos[u.id] = pos
            pos += 1
    idx_last_pos = {}
    for num, p in last_pos.items():
        i = num_to_idx[num]
        idx_last_pos[i] = max(idx_last_pos.get(i, -1), p)
    order = sorted(
        (i for i, v in enumerate(vals) if v),
        key=lambda i: idx_last_pos.get(i, -1),
    )
    for i in order:
        single = [0] * len(vals)
        single[i] = vals[i]
        nop = self.nc.gpsimd.nop(nofuse=True)
        wait_clock.add_sem_waits(
            nop.ins, _br.ScopedClock({None: _br.VectorClock(single)})
        )
    self.nc.sync.drain()
    assert self.sems is not None
    popped = self.nc._tile_sem_poison_stack.pop()
    assert popped is self._sem_poison
    self.nc.clear_and_free_semaphores(list(self.sems.allocated().values()))


tile.TileContext._drain_and_barrier = _split_drain_and_barrier

_orig_commit_and_lower = tile.TileContext._commit_and_lower

# Same walrus limitation for regular instructions: Matmult (S3_LW) takes no
# extra sync waits, most others take one. Spill excess waits onto dedicated
# same-engine nops committed immediately before the instruction.
_ZERO_WAIT_OPS = ("InstMatmult", "InstDrain")


def _split_commit_and_lower(self, inst, original_block, old_bb_map, bb_to_exit_bb):
    tn = type(inst).__name__
    if tn.startswith("Inst") and inst.engine is not None:
        si = inst.sync_info
        if si is not None:
            waits = list(si.on_wait)
            keep = 0 if tn in _ZERO_WAIT_OPS else 1
            if len(waits) > keep:
                spill, keep_waits = (
                    (waits, []) if keep == 0 else (waits[:-1], [waits[-1]])
                )
                for w_ in spill:
                    nop = mybir.InstNoOp(
                        name=self.nc.get_next_instruction_name(),
                        engine=inst.engine,
                        sync_info=mybir.SyncInfo(on_wait=[w_], on_update=[]),
                        bass_nofuse=True,
                    )
                    self._commit_instruction(nop)
                inst.sync_info = mybir.SyncInfo(
                    on_wait=keep_waits, on_update=list(si.on_update)
                )
    return _orig_commit_and_lower(self, inst, original_block, old_bb_map, bb_to_exit_bb)


tile.TileContext._commit_and_lower = _split_commit_and_lower


def build_nc() -> bass.Bass:
    # Bass.__init__ emits four const-AP memsets on the Pool engine, whose
    # 95ns-per-op GPSIMD launch delays the opening all-engine barrier (and
    # with it the first DMA) by ~130ns. Route them to the DVE for init.
    orig_memset = bass.BassGpSimd.memset

    def _dve_memset(self, ap, value):
        return self.bass.vector.memset(ap, value)

    # Also skip the init-trailing all-engine barrier: nothing in this kernel
    # reads the const APs those memsets initialize (Copy-activation bias is
    # immediate, affine_select fill is a register), and Tile's own semaphores
    # order every cross-engine dependency we do have. Skipping it lets the
    # first DMA decode immediately instead of waiting out the slowest
    # engine's init.
    orig_barrier = bass.Bass.all_engine_barrier
    bass.BassGpSimd.memset = _dve_memset
    bass.Bass.all_engine_barrier = lambda self, *a, **k: None
    try:
        nc = bass.Bass()
    finally:
        bass.BassGpSimd.memset = orig_memset
        bass.Bass.all_engine_barrier = orig_barrier
    yt_d = nc.declare_dram_parameter("yt", [N, BC], F16, isOutput=False)
    bt_d = nc.declare_dram_parameter("bt", [M, BC], F8, isOutput=False)
    # w carries [W_mixed | AA^T] fused in one upload: one 819ns DMA instead
    # of two (a sub-625ns transfer would bubble the HWDGE pipeline).
    w_d = nc.declare_dram_parameter("w", [M, N + M], F16, isOutput=False)
    out_d = nc.declare_dram_parameter("out", [N, BC], F16, isOutput=True)

    # dim-chunked 3D views: partition = row-within-chunk, then (chunk, batch)
    yt_v = yt_d.rearrange("(k p) b -> p k b", p=128)
    out_v = out_d.rearrange("(k p) b -> p k b", p=128)

    with tile.TileContext(nc) as tc:
        with (
            tc.tile_pool(name="const", bufs=1) as constp,
            tc.tile_pool(name="yts", bufs=NJ) as ytp,
            tc.tile_pool(name="tts", bufs=3) as ttp,
            tc.tile_pool(name="outs", bufs=6) as outp,
            tc.tile_pool(name="ps1", bufs=2, space="PSUM") as ps1,
            tc.tile_pool(name="ps2", bufs=6, space="PSUM") as ps2,
        ):
            # PE pstate pre-ramp: the cost of a matmul depends on how long the
            # PE has been continuously busy (LOW->MID->FULL over ~3us). Dummy
            # zero matmuls starting right after the preamble put the engine at
            # FULL speed by the time the first real operand tile lands.
            wz = constp.tile([128, 384], F16)
            nc.gpsimd.memset(wz[:], 0.0)
            warm = ps1.tile([128, F], F32, name="pt")

            def warmup(n):
                for _ in range(n):
                    nc.tensor.matmul(
                        warm[:, 0:256], wz[:, 0:128], wz[:, 128:384],
                        start=True, stop=True,
                    )

            # the pre-ramp runs while the W upload is in flight; the
            # at-derivation matmuls slot in right when it lands and keep the
            # ramp going until y0 arrives — no cycles wasted on a saturated PE
            warmup(N_WARM)

            # Load order: stage-1 operands first, so the first real matmul
            # starts ~3us earlier than a consts-first order. The A^T chunks
            # are not uploaded at all: A = (AA^T) W, so at[p,k,m] is derived
            # on-device as (W chunk)^T (AA^T) from the W matrix (needed for
            # stage 2 anyway) plus a 32KB AA^T — replacing a 728ns constant
            # load with 91ns and some matmul/copy work in the idle fill
            # window. W's odd chunks arrive negated (for the stage-2
            # PE-accumulate path), so odd-chunk derivations downcast with
            # scale -1 to restore A's sign.
            w_s = constp.tile([128, N + M], F16)  # partition = m, free = dim
            nc.sync.dma_start(w_s[:], w_d[:])
            at_s = constp.tile([128, KC, M], F16)  # A^T chunks: p=dim, free=m
            for k in range(KC):
                pd = ps2.tile([128, M], F32, name="p2")
                nc.tensor.matmul(
                    pd[:],
                    w_s[:, k * 128:(k + 1) * 128],
                    w_s[:, N:N + M],
                    start=True,
                    stop=True,
                )
                # w is fully negated, so every derived chunk flips sign
                if k % 2 == 1:
                    nc.vector.tensor_scalar_mul(at_s[:, k, :], pd[:], -1.0)
                else:
                    nc.scalar.mul(at_s[:, k, :], pd[:], -1.0)
            ytjs = []
            for j in range(NJ):
                ytj = ytp.tile([128, KC, F], F16, name=f"ytj{j}")
                ytjs.append(ytj)
            # y tile loads are split in k-halves so stage-1 k=0..3 matmuls of
            # a batch tile can begin after half its load has landed.
            def load_y(j, h):
                nc.sync.dma_start(
                    ytjs[j][:, h * 4:(h + 1) * 4, :],
                    yt_v[:, h * 4:(h + 1) * 4, j * F:(j + 1) * F],
                )
            load_y(0, 0)
            load_y(0, 1)
            bt_s = constp.tile([128, BC], F8)  # partition = m, free = batch
            nc.sync.dma_start(bt_s[:], bt_d[:])
            # 128x128 fp16 identity built in place (last tile's even chunks
            # accumulate +I^T y on the PE): ones, keep the diagonal.
            id_s = constp.tile([128, 128], F16)
            nc.gpsimd.memset(id_s[:], 1.0)
            nc.gpsimd.affine_select(
                id_s[:], id_s[:], [[1, 128]],
                mybir.AluOpType.is_equal, 0.0,
                base=0, channel_multiplier=-1,
            )
            for j in range(1, NJ):
                load_y(j, 0)
                load_y(j, 1)

            # Stage bodies. s1(j): 8 accumulating matmuls + the DVE
            # subtract/downcast that produces T in fp16. s2(j): the Act half
            # first (PE accumulates y - W^T T, Act only copies out of PSUM)
            # so the Activation engine starts early, then the DVE-subtract
            # half; each d-chunk pair streams out in its own 728ns store.
            tts = {}
            pts = {}
            ohs = {}

            def s1a(j):
                ytj = ytjs[j]
                pt = ps1.tile([128, F], F32, name="pt")
                pts[j] = pt
                for k in range(KC // 2):
                    nc.tensor.matmul(
                        pt[:],
                        at_s[:, k, :],
                        ytj[:, k, :],
                        start=(k == 0),
                        stop=False,
                        skip_group_check=True,
                    )

            def s1b(j):
                ytj = ytjs[j]
                pt = pts[j]
                for k in range(KC // 2, KC):
                    nc.tensor.matmul(
                        pt[:],
                        at_s[:, k, :],
                        ytj[:, k, :],
                        start=False,
                        stop=(k == KC - 1),
                        skip_group_check=True,
                    )
                tt = ttp.tile([128, F], F16, name="tt")
                nc.vector.tensor_sub(
                    tt[:], pt[:], bt_s[:, j * F:(j + 1) * F]
                )
                tts[j] = tt

            def s2_half(j, h):
                # Per-chunk stage 2: one PSUM bank per d-chunk (6-deep
                # rotation), consumer alternates DVE (even d: y - PSUM) and
                # Act (odd d: PE accumulates -W^T T + I^T Y, Act copies).
                # Finer grains keep every engine's idle gaps at sem latency.
                ytj = ytjs[j]
                tt = tts[j]
                if h == 0:
                    oh = outp.tile([128, KC, F], F16, name="oh")
                    ohs[j] = oh
                else:
                    oh = ohs[j]
                last = j == NJ - 1
                for d in range(h * (KC // 2), (h + 1) * (KC // 2)):
                    p2 = ps2.tile([128, F], F32, name="p2")
                    act = d % 2 == 1
                    idmm = last and not act
                    nc.tensor.matmul(
                        p2[:],
                        w_s[:, d * 128:(d + 1) * 128],
                        tt[:],
                        start=True,
                        stop=not idmm,
                    )
                    if idmm:
                        # last tile, even chunks: PE accumulates +I^T y so
                        # the Act engine can produce y - corr with a plain
                        # copy; its counterpart DVE handles the odd chunks.
                        # (Pool's ~1us serialized accum-store issue would sit
                        # in the drain path if the last tile used it.)
                        nc.tensor.matmul(
                            p2[:],
                            id_s[:],
                            ytj[:, d, :],
                            start=False,
                            stop=True,
                        )
                        nc.scalar.copy(oh[:, d, :], p2[:])
                    elif act and not last:
                        # PSUM = -corr (W is negated). Act downcasts it and a
                        # Pool SWDGE store accumulate-ADDs it onto the
                        # y^T-prefilled output DRAM — no +y engine work.
                        nc.scalar.copy(oh[:, d, :], p2[:])
                        nc.gpsimd.dma_start(
                            out_v[:, d, j * F:(j + 1) * F],
                            oh[:, d, :],
                            accum_op=mybir.AluOpType.add,
                        )
                    else:
                        # DVE computes y + (-corr) directly
                        nc.vector.tensor_add(
                            oh[:, d, :], ytj[:, d, :], p2[:]
                        )
                        if last:
                            # paired store: 364ns transfers can't be issued
                            # faster than the ~650ns SEQ+HWDGE pipe, which
                            # would stretch the drain — 728ns pairs can
                            if act:
                                nc.sync.dma_start(
                                    out_v[:, d - 1:d + 1,
                                          j * F:(j + 1) * F],
                                    oh[:, d - 1:d + 1, :],
                                )
                        else:
                            nc.sync.dma_start(
                                out_v[:, d, j * F:(j + 1) * F],
                                oh[:, d, :],
                            )

            # Half-granular software pipeline: s1 halves of a later batch
            # tile interleave with s2 halves of an earlier one, so the PE
            # never stalls on T and — equally important — chunk-pair stores
            # are produced at an even rate (an unbroken 8-matmul s1 block
            # would starve the store queue for ~1.7us and open a DMA gap).
            s1a(0)
            s1b(0)
            s2_half(0, 0)
            s1a(1)
            s2_half(0, 1)
            s1b(1)
            for j in range(2, NJ):
                s1a(j)
                s2_half(j - 1, 0)
                s1b(j)
                s2_half(j - 1, 1)
            s2_half(NJ - 1, 0)
            s2_half(NJ - 1, 1)
    return nc


_NC_CACHE = None
_RUNNER = None


def _get_nc():
    global _NC_CACHE
    if _NC_CACHE is None:
        _NC_CACHE = build_nc()
    return _NC_CACHE


def _build_runner():
    """Persistent jitted shard_map callable over 8 cores (mirrors
    bass2jax.run_bass_via_pjrt's multi-core path, but cached so repeated
    kernel() calls skip retracing/XLA recompile)."""
    import jax
    from jax.sharding import Mesh, PartitionSpec
    from jax.experimental.shard_map import shard_map
    from concourse import bass2jax as b2j

    nc = _get_nc()
    b2j.install_neuronx_cc_hook()
    assert nc.dbg_addr is None
    partition_name = nc.partition_id_tensor.name if nc.partition_id_tensor else None

    in_names, out_names, out_avals, zero_shapes = [], [], [], []
    for alloc in nc.m.functions[0].allocations:
        if not isinstance(alloc, mybir.MemoryLocationSet):
            continue
        name = alloc.memorylocations[0].name
        if alloc.kind == "ExternalInput":
            if name != partition_name:
                in_names.append(name)
        elif alloc.kind == "ExternalOutput":
            out_names.append(name)
            shape = tuple(alloc.tensor_shape)
            dtype = mybir.dt.np(alloc.dtype)
            out_avals.append(jax.core.ShapedArray(shape, dtype))
            zero_shapes.append((shape, dtype))
    n_params = len(in_names)
    n_outs = len(out_names)
    all_in_names = tuple(in_names) + tuple(out_names)
    if partition_name is not None:
        all_in_names = all_in_names + (partition_name,)

    def _body(*args):
        operands = list(args)
        if partition_name is not None:
            operands.append(b2j.partition_id_tensor())
        outs = b2j._bass_exec_p.bind(
            *operands,
            out_avals=tuple(out_avals),
            in_names=all_in_names,
            out_names=tuple(out_names),
            lowering_input_output_aliases=(),
            sim_require_finite=True,
            sim_require_nnan=True,
            nc=nc,
        )
        return tuple(outs)

    devices = jax.devices()[:N_CORES]
    mesh = Mesh(np.asarray(devices), ("core",))
    in_specs = (PartitionSpec("core"),) * (n_params + n_outs)
    out_specs = (PartitionSpec("core"),) * n_outs
    donate = tuple(range(n_params, n_params + n_outs))
    sharded = jax.jit(
        shard_map(
            _body, mesh=mesh, in_specs=in_specs, out_specs=out_specs,
            check_rep=False,
        ),
        donate_argnums=donate,
        keep_unused=True,
    )

    del zero_shapes

    def make_inits(named_inputs: dict):
        # The out buffer is uploaded as the kernel's initial output DRAM
        # content: it starts as y^T, and the odd d-chunk stores
        # accumulate-ADD their -corr contribution onto it.
        return [np.ascontiguousarray(named_inputs["yt"])]

    def run(named_inputs: dict):
        """named_inputs: name -> concatenated (N_CORES*dim0, ...) array."""
        ins = [named_inputs[n] for n in in_names]
        outs = sharded(*ins, *make_inits(named_inputs))
        return dict(zip(out_names, outs))

    run._parts = {
        "sharded": sharded,
        "in_names": in_names,
        "out_names": out_names,
        "mesh": mesh,
        "make_inits": make_inits,
    }
    return run


def _get_runner():
    global _RUNNER
    if _RUNNER is None:
        _RUNNER = _build_runner()
    return _RUNNER


def _prep_inputs(y, A, b):
    f16 = _np_f16()
    A64 = A.astype(np.float64)
    W = np.linalg.solve(A64 @ A64.T, A64)  # (M, N)
    # W uploads fully negated: every stage-2 PSUM holds -corr, so DVE
    # consumers add y, Act consumers copy for the accumulate-ADD stores,
    # and the last tile's identity-matmul path accumulates +I^T y on top.
    W_mixed = (-W).astype(f16)
    # device rebuilds A^T chunks as W^T(AA^T); AA^T rides inside the W upload
    wext = np.concatenate([W_mixed, (A64 @ A64.T).astype(f16)], axis=1)
    f8 = np.dtype(mybir.dt.np(F8))
    y16 = y.astype(f16)
    b16 = b.astype(f8)
    # concat-over-cores layouts expected by the shard_map runner
    yt_cat = np.ascontiguousarray(
        y16.reshape(N_CORES, BC, N).transpose(0, 2, 1)
    ).reshape(N_CORES * N, BC)
    bt_cat = np.ascontiguousarray(
        b16.reshape(N_CORES, BC, M).transpose(0, 2, 1)
    ).reshape(N_CORES * M, BC)
    w_cat = np.broadcast_to(wext, (N_CORES, M, N + M)).reshape(
        N_CORES * M, N + M
    )
    return {"yt": yt_cat, "bt": bt_cat, "w": w_cat}


def _unpack_output(out_cat: np.ndarray) -> np.ndarray:
    return (
        np.asarray(out_cat)
        .reshape(N_CORES, N, BC)
        .transpose(0, 2, 1)
        .astype(np.float32)
        .reshape(BATCH, N)
    )


def kernel(y: np.ndarray, A: np.ndarray, b: np.ndarray) -> np.ndarray:
    y = np.ascontiguousarray(np.asarray(y, dtype=np.float32))
    A = np.ascontiguousarray(np.asarray(A, dtype=np.float32))
    b = np.ascontiguousarray(np.asarray(b, dtype=np.float32))
    assert y.shape == (BATCH, N) and A.shape == (M, N) and b.shape == (BATCH, M)

    named = _prep_inputs(y, A, b)
    try:
        run = _get_runner()
        out = run(named)["out"]
        return _unpack_output(out)
    except Exception:
        # Fallback: slower but uses only the public SPMD entry point. It
        # cannot prefill the output DRAM with y^T, so the odd d-chunks come
        # back as -corr alone and the y contribution is restored here.
        in_maps = [
            {
                k: np.ascontiguousarray(
                    v.reshape(N_CORES, v.shape[0] // N_CORES, *v.shape[1:])[i]
                )
                for k, v in named.items()
            }
            for i in range(N_CORES)
        ]
        res = run_bass_kernel_spmd(_get_nc(), in_maps, list(range(N_CORES)))
        x = np.empty((BATCH, N), dtype=np.float32)
        odd = (np.arange(N) // 128) % 2 == 1
        for i in range(N_CORES):
            xi = np.asarray(res.results[i]["out"]).T.astype(np.float32)
            yi = in_maps[i]["yt"].T.astype(np.float32)
            # last batch tile's odd chunks were completed on-device
            rows = slice(0, (NJ - 1) * F)
            xi[rows, odd] += yi[rows, odd]
            x[i * BC:(i + 1) * BC, :] = xi
        return x
